# revision 8
# baseline (speedup 1.0000x reference)
import numpy as np

B, S, D, F, E = 2, 2048, 1024, 4096, 8
T = B * S
K_TOP = 2
C = 1152
P = 128
PASSES = [(0, 640), (640, 512)]

_CACHE = {}


def _build_program(loop_n=1, reps=1):
    import concourse.bass as bass
    import concourse.mybir as mybir
    import concourse.tile as tile
    from concourse import bacc
    from contextlib import ExitStack

    f32 = mybir.dt.float32
    f32r = mybir.dt.float32r
    i32 = mybir.dt.int32

    nc = bacc.Bacc("TRN2", target_bir_lowering=False, debug=False)

    x_d = nc.dram_tensor("x", [T, D], f32r, kind="ExternalInput").ap()
    w1_d = nc.dram_tensor("W1", [D, F], f32r, kind="ExternalInput").ap()
    w2_d = nc.dram_tensor("W2", [F, D], f32r, kind="ExternalInput").ap()
    idx_d = nc.dram_tensor("idx", [P, C // P], i32, kind="ExternalInput").ap()
    wc_d = nc.dram_tensor("wc", [P, C // P], f32, kind="ExternalInput").ap()
    b1_d = nc.dram_tensor("b1t", [P, F // P], f32, kind="ExternalInput").ap()
    b2_d = nc.dram_tensor("b2t", [P, D // P], f32, kind="ExternalInput").ap()
    idr_d = nc.dram_tensor("identr", [P, P], f32r, kind="ExternalInput").ap()
    idf_d = nc.dram_tensor("identf", [P, P], f32, kind="ExternalInput").ap()
    y_d = nc.dram_tensor("yout", [C, D], f32, kind="ExternalOutput").ap()

    KD = D // P
    NJ = F // P
    NI = D // P

    with tile.TileContext(nc) as tc, ExitStack() as ctx:
        sb = ctx.enter_context(tc.tile_pool(name="sb", bufs=1))
        ps = ctx.enter_context(tc.tile_pool(name="ps", bufs=1, space="PSUM"))

        idx_t = sb.tile([P, C // P], i32, tag="idx")
        wc_t = sb.tile([P, C // P], f32, tag="wc")
        b1_t = sb.tile([P, F // P], f32, tag="b1")
        b2_t = sb.tile([P, D // P], f32, tag="b2")
        idr_t = sb.tile([P, P], f32r, tag="idr")
        idf_t = sb.tile([P, P], f32, tag="idf")
        nc.sync.dma_start(idx_t[:], idx_d[:])
        nc.sync.dma_start(wc_t[:], wc_d[:])
        nc.sync.dma_start(b1_t[:], b1_d[:])
        nc.sync.dma_start(b2_t[:], b2_d[:])
        nc.sync.dma_start(idr_t[:], idr_d[:])
        nc.sync.dma_start(idf_t[:], idf_d[:])

        loop_cm = tc.For_i(0, loop_n, 1) if loop_n > 1 else None
        if loop_cm is not None:
            loop_cm.__enter__()

        for rep, (t0, TP) in [(r, p) for r in range(reps) for p in PASSES]:
            NT = TP // P
            NS = TP // 2
            g0 = t0 // P

            xT = sb.tile([P, KD * TP], f32r, tag="xT", bufs=1,
                         name=f"xT_{t0}")
            for g in range(NT):
                xg = sb.tile([P, D], f32r, tag="xg", bufs=3, name=f"xg_{t0}_{g}")
                nc.gpsimd.indirect_dma_start(
                    out=xg[:], out_offset=None,
                    in_=x_d[:],
                    in_offset=bass.IndirectOffsetOnAxis(
                        ap=idx_t[:, g0 + g:g0 + g + 1], axis=0),
                )
                for k in range(KD):
                    tp = ps.tile([P, P], f32r, tag="tp", bufs=2,
                                 name=f"tpx_{t0}_{g}_{k}")
                    nc.tensor.transpose(
                        out=tp[:], in_=xg[:, k * P:(k + 1) * P],
                        identity=idr_t[:])
                    nc.vector.tensor_copy(
                        xT[:, k * TP + g * P: k * TP + (g + 1) * P], tp[:])

            h = sb.tile([P, NJ * TP], f32r, tag="h", bufs=1, name=f"h_{t0}")
            for j2 in range(NJ // 2):
                acc = [[ps.tile([P, NS], f32, tag="mm", bufs=6,
                                name=f"p1_{t0}_{j2}_{jj}_{n}")
                        for n in range(2)] for jj in range(2)]
                for k in range(KD):
                    w1t = sb.tile([P, 2 * P], f32r, tag="w1", bufs=6,
                                  name=f"w1_{t0}_{j2}_{k}")
                    nc.sync.dma_start(
                        w1t[:],
                        w1_d[k * P:(k + 1) * P, j2 * 2 * P:(j2 + 1) * 2 * P])
                    for jj in range(2):
                        for n in range(2):
                            nc.tensor.matmul(
                                acc[jj][n][:],
                                lhsT=w1t[:, jj * P:(jj + 1) * P],
                                rhs=xT[:, k * TP + n * NS:
                                       k * TP + (n + 1) * NS],
                                start=(k == 0), stop=(k == KD - 1))
                for jj in range(2):
                    j = j2 * 2 + jj
                    for n in range(2):
                        nc.scalar.activation(
                            h[:, j * TP + n * NS: j * TP + (n + 1) * NS],
                            acc[jj][n][:],
                            mybir.ActivationFunctionType.Relu,
                            bias=b1_t[:, j:j + 1])

            o2 = sb.tile([P, NI * TP], f32, tag="o2", bufs=1, name=f"o2_{t0}")
            w2_r = w2_d.rearrange("(j p) d -> p j d", p=P)
            for i in range(NI):
                w2s = sb.tile([P, NJ * P], f32r, tag="w2", bufs=2,
                              name=f"w2_{t0}_{i}")
                nc.sync.dma_start(
                    w2s[:].rearrange("p (j c) -> p j c", c=P),
                    w2_r[:, :, i * P:(i + 1) * P])
                acc2 = [ps.tile([P, NS], f32, tag="mm", bufs=6,
                                name=f"p2_{t0}_{i}_{n}") for n in range(2)]
                for j in range(NJ):
                    for n in range(2):
                        nc.tensor.matmul(
                            acc2[n][:],
                            lhsT=w2s[:, j * P:(j + 1) * P],
                            rhs=h[:, j * TP + n * NS: j * TP + (n + 1) * NS],
                            start=(j == 0), stop=(j == NJ - 1))
                for n in range(2):
                    nc.vector.tensor_scalar_add(
                        o2[:, i * TP + n * NS: i * TP + (n + 1) * NS],
                        acc2[n][:], b2_t[:, i:i + 1])

            for g in range(NT):
                ot = sb.tile([P, D], f32, tag="ot", bufs=2,
                             name=f"ot_{t0}_{g}")
                for i in range(NI):
                    tp2 = ps.tile([P, P], f32, tag="tp", bufs=2,
                                  name=f"tpo_{t0}_{g}_{i}")
                    nc.tensor.transpose(
                        out=tp2[:], in_=o2[:, i * TP + g * P: i * TP + (g + 1) * P],
                        identity=idf_t[:])
                    nc.vector.tensor_scalar_mul(
                        ot[:, i * P:(i + 1) * P], tp2[:],
                        wc_t[:, g0 + g:g0 + g + 1])
                nc.sync.dma_start(
                    y_d[t0 + g * P: t0 + (g + 1) * P, :], ot[:])

        if loop_cm is not None:
            loop_cm.__exit__(None, None, None)

    nc.compile()
    return nc


def _route(x2, Wg, bg):
    gate = x2.astype(np.float64) @ np.asarray(Wg, np.float64) + np.asarray(bg, np.float64)
    part = np.argpartition(-gate, K_TOP - 1, axis=1)[:, :K_TOP]
    rows = np.arange(T)[:, None]
    sc = gate[rows, part]
    sc = sc - sc.max(axis=1, keepdims=True)
    e_sc = np.exp(sc)
    probs = e_sc / e_sc.sum(axis=1, keepdims=True)
    idx_e, w_e, n_e = [], [], []
    for e in range(E):
        mask = part == e
        tok = np.nonzero(mask.any(axis=1))[0]
        pr = probs[mask]
        n = len(tok)
        pad = C - n
        if pad < 0:
            return None
        idx_e.append(np.concatenate([tok, np.zeros(pad, np.int64)]).astype(np.int32))
        w_e.append(np.concatenate([pr, np.zeros(pad)]).astype(np.float32))
        n_e.append(n)
    return idx_e, w_e, n_e


def _numpy_fallback(x2, W1, b1, W2, b2, routing):
    idx_e, w_e, n_e = routing
    out = np.zeros((T, D), np.float32)
    for e in range(E):
        n = n_e[e]
        tok = idx_e[e][:n]
        hcur = np.maximum(x2[tok] @ np.asarray(W1[e]) + np.asarray(b1[e]), 0.0)
        y = hcur @ np.asarray(W2[e]) + np.asarray(b2[e])
        out[tok] += w_e[e][:n, None] * y
    return out


def kernel(x, W1, b1, W2, b2, Wg, bg, num_experts_per_token):
    from concourse.bass_utils import run_bass_kernel_spmd

    x2 = np.asarray(x, np.float32).reshape(T, D)
    W1 = np.asarray(W1, np.float32)
    b1 = np.asarray(b1, np.float32)
    W2 = np.asarray(W2, np.float32)
    b2 = np.asarray(b2, np.float32)

    routing = _route(x2, Wg, bg)
    if routing is None or int(num_experts_per_token) != K_TOP:
        gate = x2.astype(np.float64) @ np.asarray(Wg, np.float64) + np.asarray(bg, np.float64)
        k = int(num_experts_per_token)
        part = np.argsort(-gate, axis=1)[:, :k]
        sc = gate[np.arange(T)[:, None], part]
        sc = sc - sc.max(axis=1, keepdims=True)
        pr = np.exp(sc); pr /= pr.sum(axis=1, keepdims=True)
        out = np.zeros((T, D), np.float32)
        for e in range(E):
            mask = part == e
            tok = np.nonzero(mask.any(axis=1))[0]
            w = pr[mask].astype(np.float32)
            hcur = np.maximum(x2[tok] @ W1[e] + b1[e], 0.0)
            out[tok] += w[:, None] * (hcur @ W2[e] + b2[e])
        return out.reshape(B, S, D)

    idx_e, w_e, n_e = routing

    if "nc" not in _CACHE:
        _CACHE["nc"] = _build_program()
    nc = _CACHE["nc"]

    ident = np.eye(P, dtype=np.float32)
    in_maps = []
    for e in range(E):
        in_maps.append({
            "x": x2,
            "W1": W1[e],
            "W2": W2[e],
            "idx": np.ascontiguousarray(idx_e[e].reshape(C // P, P).T),
            "wc": np.ascontiguousarray(w_e[e].reshape(C // P, P).T),
            "b1t": np.ascontiguousarray(b1[e].reshape(F // P, P).T),
            "b2t": np.ascontiguousarray(b2[e].reshape(D // P, P).T),
            "identr": ident,
            "identf": ident,
        })

    res = run_bass_kernel_spmd(nc, in_maps, list(range(E)))

    out = np.zeros((T, D), np.float32)
    for e in range(E):
        n = n_e[e]
        out[idx_e[e][:n]] += res.results[e]["yout"][:n]
    return out.reshape(B, S, D)
